# revision 15
# baseline (speedup 1.0000x reference)
"""Trainium2 Bass kernel for nn_Blur (upfirdn2d 4x4 blur, pad=(2,1)).

Formulation: out[i,j] = sum_{p,q} Kf[p,q] * x[i+p-2, j+q-2]   (Kf = flip(kernel2d))

For each W-tap q (4 taps), the H-convolution is a banded 64x64 matrix
Aq[i,h] = Kf[h-i+2, q].  x is split host-side into bf16 hi + bf16 lo
(x = hi + lo to ~2^-18 relative).  The blur weights ({1,3,9}/64) have <=4
mantissa bits, so every bf16 product is exact in fp32: the PSUM accumulation
reproduces the fp32 conv to ~1e-6 while streaming at bf16 rate.

One image's hi rows (partitions 0-63) and lo rows (64-127) fill the full
K=128 contraction: lhsT_q = [Aq^T; Aq^T] [128, 64] computes Aq@(hi+lo) in a
single matmul with M=64.  Two such matmuls (8 images each via a stride-68
N=512 access pattern) run CONCURRENTLY on disjoint PE column groups
(tile_position (0,0) / (0,64)), halving effective PE time.  The 4 taps
accumulate into one PSUM bank.

DMA: the host pre-transposes into per-batch [128, 1092] bf16 tiles
(partition-major contiguous, 68-stride zero-padded rows so tap windows read
zeros at W boundaries), so every DMA is a plain 128-descriptor line-rate
transfer.  The fp32 output is written back as [128, 512] tiles and
inverse-transposed on the host.

Sharding: the 16*512 = 8192 independent (n,c) images are split into 8
contiguous slabs of 1024 images, one per NeuronCore (data-parallel).
"""

import ml_dtypes
import numpy as np

import concourse.bacc as bacc
import concourse.bass as bass
import concourse.mybir as mybir
import concourse.tile as tile
from concourse.bass_utils import run_bass_kernel_spmd

N_CORES = 8
IMG = 64                      # H = W
N_IMAGES = 16 * 512           # 8192
PER_CORE = N_IMAGES // N_CORES  # 1024
GROUP = 16                    # images per batch
N_BATCH = PER_CORE // GROUP   # 64
PAD_L, PAD_R = 2, 2           # row padding -> stride 68
S = PAD_L + IMG + PAD_R       # 68
HALF_W = 8 * S                # 544 cols per col-group (8 images)
TILE_W = 2 * HALF_W + 4       # 1092 (4 slack: tap q=3 slice bound)
DT = mybir.dt.float32
IN_DT = mybir.dt.bfloat16
NP_IN = ml_dtypes.bfloat16

LAST_RESULTS = None  # BassKernelResults of the most recent run (for test.py)


def _build_weights(kernel2d: np.ndarray) -> np.ndarray:
    """[128, 256] bf16: cols [64q:64q+64] hold [Aq^T; Aq^T] (hi rows; lo rows)."""
    kf = np.flip(np.asarray(kernel2d, dtype=np.float64), (0, 1))
    wts = np.zeros((128, 256), dtype=NP_IN)
    for q in range(4):
        aq = np.zeros((64, 64), dtype=np.float64)
        for i in range(64):
            for p in range(4):
                h = i + p - 2
                if 0 <= h < 64:
                    aq[i, h] = kf[p, q]
        wts[:64, q * 64:(q + 1) * 64] = aq.T.astype(NP_IN)
        wts[64:, q * 64:(q + 1) * 64] = aq.T.astype(NP_IN)
    return wts


def _bass_module() -> bass.Bass:
    nc = bacc.Bacc(
        "TRN2",
        target_bir_lowering=False,
        debug=False,
        num_devices=N_CORES,
    )
    x_d = nc.dram_tensor("x", [N_BATCH, 128, TILE_W], IN_DT, kind="ExternalInput")
    w_d = nc.dram_tensor("wts", [128, 256], IN_DT, kind="ExternalInput")
    o_d = nc.dram_tensor("out", [N_BATCH, 128, 512], DT, kind="ExternalOutput")

    with tile.TileContext(nc) as tc:
        with (
            tc.tile_pool(name="const", bufs=1) as cpool,
            tc.tile_pool(name="inp", bufs=10) as ipool,
            tc.tile_pool(name="outp", bufs=8) as opool,
            tc.tile_pool(name="psum", bufs=8, space="PSUM") as ppool,
        ):
            w_tile = cpool.tile([128, 256], IN_DT)
            nc.sync.dma_start(w_tile[:], w_d[:])

            for b in range(N_BATCH):
                in_tile = ipool.tile([128, TILE_W], IN_DT)
                nc.sync.dma_start(in_tile[:], x_d[b])

                ps = ppool.tile([128, 512], DT)
                for q in range(4):
                    for cg in range(2):  # concurrent PE column groups
                        base = cg * HALF_W + q
                        rhs = in_tile[:, base:base + 8 * S].rearrange(
                            "p (g s) -> p g s", s=S
                        )[:, :, 0:IMG]
                        nc.tensor.matmul(
                            ps[cg * 64:(cg + 1) * 64, :],
                            w_tile[:, q * 64:(q + 1) * 64],
                            rhs,
                            start=(q == 0),
                            stop=(q == 3),
                            tile_position=(0, cg * 64),
                        )

                out_tile = opool.tile([128, 512], DT)
                nc.vector.tensor_copy(out_tile[:], ps[:])
                nc.scalar.dma_start(o_d[b], out_tile[:])
    nc.compile()
    return nc


def _host_pack(x: np.ndarray) -> np.ndarray:
    """FULL x (8192,64,64) f32 -> [N_CORES, N_BATCH, 128, TILE_W] bf16.

    Partition dim = (half, h); free dim = (g: 16 images, s: 68)."""
    hi = x.astype(NP_IN)
    lo = (x - hi.astype(np.float32)).astype(NP_IN)
    packed = np.zeros((2, N_IMAGES, IMG, S), dtype=NP_IN)
    packed[0, :, :, PAD_L:PAD_L + IMG] = hi
    packed[1, :, :, PAD_L:PAD_L + IMG] = lo
    v = packed.reshape(2, N_CORES, N_BATCH, GROUP, IMG, S)
    v = v.transpose(1, 2, 0, 4, 3, 5)  # [core, b, half, h, g, s]
    flat = v.reshape(N_CORES, N_BATCH, 128, GROUP * S)
    out = np.zeros((N_CORES, N_BATCH, 128, TILE_W), dtype=NP_IN)
    out[..., : GROUP * S] = flat
    return out


def _host_unpack(tiles: np.ndarray) -> np.ndarray:
    """[N_CORES, N_BATCH, 128, 512] f32 -> (8192, 64, 64) f32.

    Partition dim = (cg, h); free dim = (g: 8, w); img = b*16 + cg*8 + g."""
    v = tiles.reshape(N_CORES, N_BATCH, 2, IMG, 8, IMG)
    v = v.transpose(0, 1, 2, 4, 3, 5)  # [core, b, cg, g, h, w]
    return v.reshape(N_IMAGES, IMG, IMG)


def kernel(x: np.ndarray, kernel: np.ndarray, _trace: bool = False) -> np.ndarray:
    global LAST_RESULTS
    x = np.ascontiguousarray(np.asarray(x, dtype=np.float32))
    n, c, h, w = x.shape
    assert (n, c, h, w) == (16, 512, 64, 64), x.shape

    shards = _host_pack(x.reshape(N_IMAGES, IMG, IMG))
    wts = _build_weights(kernel)
    in_maps = [{"x": shards[i], "wts": wts} for i in range(N_CORES)]

    nc = _bass_module()
    results = run_bass_kernel_spmd(
        nc, in_maps, core_ids=list(range(N_CORES)), trace=_trace
    )
    LAST_RESULTS = results

    tiles = np.stack([r["out"] for r in results.results])
    out = _host_unpack(tiles)
    return np.ascontiguousarray(out.reshape(n, c, h, w)).astype(np.float32)


# revision 16
# speedup vs baseline: 1.1200x; 1.1200x over previous
"""Trainium2 Bass kernel for nn_Blur (upfirdn2d 4x4 blur, pad=(2,1)).

Formulation: out[i,j] = sum_{p,q} Kf[p,q] * x[i+p-2, j+q-2]   (Kf = flip(kernel2d))

For each W-tap q (4 taps), the H-convolution is a banded 64x64 matrix
Aq[i,h] = Kf[h-i+2, q].  x is split host-side into bf16 hi + bf16 lo
(x = hi + lo to ~2^-18 relative).  The blur weights ({1,3,9}/64) have <=4
mantissa bits, so every bf16 product is exact in fp32: the PSUM accumulation
reproduces the fp32 conv to ~1e-6 while streaming at bf16 rate.

One image's hi rows (partitions 0-63) and lo rows (64-127) fill the full
K=128 contraction: lhsT_q = [Aq^T; Aq^T] [128, 64] computes Aq@(hi+lo) in a
single matmul with M=64.  Two such matmuls (8 images each via a stride-68
N=512 access pattern) run CONCURRENTLY on disjoint PE column groups
(tile_position (0,0) / (0,64)), halving effective PE time.  The 4 taps
accumulate into one PSUM bank.

DMA: the host pre-transposes into per-batch [128, 1092] bf16 tiles
(partition-major contiguous, 68-stride zero-padded rows so tap windows read
zeros at W boundaries), so every DMA is a plain 128-descriptor line-rate
transfer.  The fp32 output is written back as [128, 512] tiles and
inverse-transposed on the host.

Sharding: the 16*512 = 8192 independent (n,c) images are split into 8
contiguous slabs of 1024 images, one per NeuronCore (data-parallel).
"""

import ml_dtypes
import numpy as np

import concourse.bacc as bacc
import concourse.bass as bass
import concourse.mybir as mybir
import concourse.tile as tile
from concourse.bass_utils import run_bass_kernel_spmd

N_CORES = 8
IMG = 64                      # H = W
N_IMAGES = 16 * 512           # 8192
PER_CORE = N_IMAGES // N_CORES  # 1024
GROUP = 32                    # images per batch (4 chunks of 8)
N_BATCH = PER_CORE // GROUP   # 32
PAD_L, PAD_R = 2, 2           # row padding -> stride 68
S = PAD_L + IMG + PAD_R       # 68
HALF_W = 8 * S                # 544 cols per 8-image chunk
TILE_W = 4 * HALF_W + 4       # 2180 (4 slack: tap q=3 slice bound)
DT = mybir.dt.float32
IN_DT = mybir.dt.bfloat16
NP_IN = ml_dtypes.bfloat16

LAST_RESULTS = None  # BassKernelResults of the most recent run (for test.py)


def _build_weights(kernel2d: np.ndarray) -> np.ndarray:
    """[128, 256] bf16: cols [64q:64q+64] hold [Aq^T; Aq^T] (hi rows; lo rows)."""
    kf = np.flip(np.asarray(kernel2d, dtype=np.float64), (0, 1))
    wts = np.zeros((128, 256), dtype=NP_IN)
    for q in range(4):
        aq = np.zeros((64, 64), dtype=np.float64)
        for i in range(64):
            for p in range(4):
                h = i + p - 2
                if 0 <= h < 64:
                    aq[i, h] = kf[p, q]
        wts[:64, q * 64:(q + 1) * 64] = aq.T.astype(NP_IN)
        wts[64:, q * 64:(q + 1) * 64] = aq.T.astype(NP_IN)
    return wts


def _bass_module() -> bass.Bass:
    nc = bacc.Bacc(
        "TRN2",
        target_bir_lowering=False,
        debug=False,
        num_devices=N_CORES,
    )
    x_d = nc.dram_tensor("x", [N_BATCH, 128, TILE_W], IN_DT, kind="ExternalInput")
    w_d = nc.dram_tensor("wts", [128, 256], IN_DT, kind="ExternalInput")
    o_d = nc.dram_tensor("out", [N_BATCH, 128, 1024], DT, kind="ExternalOutput")

    with tile.TileContext(nc) as tc:
        with (
            tc.tile_pool(name="const", bufs=1) as cpool,
            tc.tile_pool(name="inp", bufs=10) as ipool,
            tc.tile_pool(name="outp", bufs=8) as opool,
            tc.tile_pool(name="psum", bufs=8, space="PSUM") as ppool,
        ):
            w_tile = cpool.tile([128, 256], IN_DT)
            nc.sync.dma_start(w_tile[:], w_d[:])

            for b in range(N_BATCH):
                in_tile = ipool.tile([128, TILE_W], IN_DT)
                nc.sync.dma_start(in_tile[:], x_d[b])

                out_tile = opool.tile([128, 1024], DT)
                for hb in range(2):  # two 16-image half-batches
                    ps = ppool.tile([128, 512], DT, tag="ps")
                    for q in range(4):
                        for cg in range(2):  # concurrent PE column groups
                            base = (hb * 2 + cg) * HALF_W + q
                            rhs = in_tile[:, base:base + 8 * S].rearrange(
                                "p (g s) -> p g s", s=S
                            )[:, :, 0:IMG]
                            nc.tensor.matmul(
                                ps[cg * 64:(cg + 1) * 64, :],
                                w_tile[:, q * 64:(q + 1) * 64],
                                rhs,
                                start=(q == 0),
                                stop=(q == 3),
                                tile_position=(0, cg * 64),
                            )
                    nc.vector.tensor_copy(
                        out_tile[:, hb * 512:(hb + 1) * 512], ps[:])
                nc.scalar.dma_start(o_d[b], out_tile[:])
    nc.compile()
    return nc


def _host_pack(x: np.ndarray) -> np.ndarray:
    """FULL x (8192,64,64) f32 -> [N_CORES, N_BATCH, 128, TILE_W] bf16.

    Partition dim = (half, h); free dim = (g: 16 images, s: 68)."""
    hi = x.astype(NP_IN)
    lo = (x - hi.astype(np.float32)).astype(NP_IN)
    packed = np.zeros((2, N_IMAGES, IMG, S), dtype=NP_IN)
    packed[0, :, :, PAD_L:PAD_L + IMG] = hi
    packed[1, :, :, PAD_L:PAD_L + IMG] = lo
    v = packed.reshape(2, N_CORES, N_BATCH, 4, 8, IMG, S)
    v = v.transpose(1, 2, 0, 5, 3, 4, 6)  # [core, b, half, h, c, g8, s]
    flat = v.reshape(N_CORES, N_BATCH, 128, GROUP * S)
    out = np.zeros((N_CORES, N_BATCH, 128, TILE_W), dtype=NP_IN)
    out[..., : GROUP * S] = flat
    return out


def _host_unpack(tiles: np.ndarray) -> np.ndarray:
    """[N_CORES, N_BATCH, 128, 1024] f32 -> (8192, 64, 64) f32.

    Partition dim = (cg, h); free dim = (g: 8, w); img = b*16 + cg*8 + g."""
    v = tiles.reshape(N_CORES, N_BATCH, 2, IMG, 2, 8, IMG)
    # [core, b, cg, h, hb, g8, w] -> [core, b, hb, cg, g8, h, w]
    v = v.transpose(0, 1, 4, 2, 5, 3, 6)
    return v.reshape(N_IMAGES, IMG, IMG)


def kernel(x: np.ndarray, kernel: np.ndarray, _trace: bool = False) -> np.ndarray:
    global LAST_RESULTS
    x = np.ascontiguousarray(np.asarray(x, dtype=np.float32))
    n, c, h, w = x.shape
    assert (n, c, h, w) == (16, 512, 64, 64), x.shape

    shards = _host_pack(x.reshape(N_IMAGES, IMG, IMG))
    wts = _build_weights(kernel)
    in_maps = [{"x": shards[i], "wts": wts} for i in range(N_CORES)]

    nc = _bass_module()
    results = run_bass_kernel_spmd(
        nc, in_maps, core_ids=list(range(N_CORES)), trace=_trace
    )
    LAST_RESULTS = results

    tiles = np.stack([r["out"] for r in results.results])
    out = _host_unpack(tiles)
    return np.ascontiguousarray(out.reshape(n, c, h, w)).astype(np.float32)
